# revision 19
# baseline (speedup 1.0000x reference)
"""Trainium2 Bass kernel for the NCA-style dense CNN problem (v3).

Math (per batch image):
  y    = perception(x)  : 4 fixed 3x3 filters per channel, circular pad
  hid  = relu(w1 @ y + b1)   (1x1 conv 32->16)
  delta= w2 @ hid            (1x1 conv 16->8)
  out  = x + delta * mask

Host-side folding: perception + w1 collapse into one 3x3 conv with weights
Weff[o,c,dy,dx] = sum_f w1[o,4c+f] * filt[f,dy,dx].  Stage 1 is evaluated as
the raw 9-tap conv in fp8e4m3 using DoubleRow matmuls: each DoubleRow mm
contracts an extra pair dimension taken from the rhs AP, so two taps ride in
one matmul (pair = adjacent rows).  dy in {-1,0} pair naturally per dx;
the dy=+1 row pairs with the next row under a zero weight plane (overlapping
column-pair APs wedge the hardware).  9 taps -> 6 DoubleRow matmuls at
0.5 cycles/col.  Per-hid-channel fp8 scale s_o is folded into w2 and the
relu bias.

Device mapping (per core: 2 batch images, batch-sharded across 8 cores):
  - strip = 256 rows; xb [128 part = (16 blk x 8 ch), 18 x 514] bf16
    (1 halo row each side, wrap cols), SWDGE cast-DMA from fp32 HBM.
  - xf8: fp8 e4m3 copy of xb on GPSIMD (+ zeroed pad row 19).
  - stage 1: 5 DoubleRow tap matmuls per block-half into PSUM
    [128 = (8 blk x 16 hid), 2, 512] (both halves, bank-aligned).
  - relu+bias on ACT -> bf16 [128, 2, 512].
  - stage 2: block-diag bf16 [128,64] matmul, halves packed into one PSUM
    bank via col-group tile_position.
  - mask: SWDGE cast-load i32 -> fp8 {0,1} grouped [16 part = blk, 16x512],
    then partition-aligned SBUF->SBUF DMA broadcast across the 8 channel
    partitions (8 HWDGE copies, one per channel).
  - epilogue: DVE mul (delta*mask, PSUM x SBUF -> bf16), DVE add (+x, all
    bf16 2x mode).  Output stored bf16; host upcasts to fp32.
"""

import numpy as np
import ml_dtypes

B, C, H, W, HID = 16, 8, 512, 512, 16
NCORES = 8
BPC = B // NCORES          # batches per core
NBLK = 16                  # row-blocks per strip
RB = 16                    # rows per block
STRIP = NBLK * RB          # 256 rows
NSTRIP = H // STRIP        # 2 strips per image

_CACHE = {}


def _fixed_filters():
    ident = np.zeros((3, 3), np.float64)
    ident[1, 1] = 1.0
    sx = np.array([[-1.0, 0.0, 1.0], [-2.0, 0.0, 2.0], [-1.0, 0.0, 1.0]]) / 8.0
    lap = np.array([[1.0, 2.0, 1.0], [2.0, -12.0, 2.0], [1.0, 2.0, 1.0]]) / 16.0
    return np.stack([ident, sx, sx.T, lap])  # [4,3,3]


# 6 DoubleRow matmuls covering the 9 taps, all with the row-pair AP shape
# (overlapping column-pair APs wedge the hardware AP walker):
#   t0-t2: pair = rows (dy=-1, dy=0), one per dx
#   t3-t5: pair = rows (dy=+1, zeroed pad row 18), one per dx (2nd weight 0)
def _tap_pairs():
    # returns list of (row_offset, col_offset, [(dy,dx,frac), (dy,dx,frac)|None])
    return [
        (0, 0, [(0, 0, 1.0), (1, 0, 1.0)]),
        (0, 1, [(0, 1, 1.0), (1, 1, 1.0)]),
        (0, 2, [(0, 2, 1.0), (1, 2, 1.0)]),
        (2, 0, [(2, 0, 1.0), None]),
        (2, 1, [(2, 1, 1.0), None]),
        (2, 2, [(2, 2, 1.0), None]),
    ]


def _build_bass():
    import concourse.mybir as mybir
    from concourse import bacc, tile

    f32 = mybir.dt.float32
    bf16 = mybir.dt.bfloat16
    fp8 = mybir.dt.float8e4
    i32 = mybir.dt.int32
    Relu = mybir.ActivationFunctionType.Relu
    DR = mybir.MatmulPerfMode.DoubleRow

    nc = bacc.Bacc(None, target_bir_lowering=False)
    x_d = nc.dram_tensor("x", (BPC, C, H, W), f32, kind="ExternalInput")
    m_d = nc.dram_tensor("update_mask", (BPC, 1, H, W), i32, kind="ExternalInput")
    w1_d = nc.dram_tensor("w1dr", (128, 6, 2, 128), fp8, kind="ExternalInput")
    w2t_d = nc.dram_tensor("w2t", (128, 64), bf16, kind="ExternalInput")
    b1_d = nc.dram_tensor("bias1", (128, 1), f32, kind="ExternalInput")
    o_d = nc.dram_tensor("out", (BPC, C, H, W), bf16, kind="ExternalOutput")

    with tile.TileContext(nc) as tc:
        with (
            tc.tile_pool(name="consts", bufs=1) as cpool,
            tc.tile_pool(name="xb", bufs=3) as xbpool,
            tc.tile_pool(name="xf8", bufs=3) as x8pool,
            tc.tile_pool(name="mk", bufs=3) as mkpool,
            tc.tile_pool(name="mfb", bufs=3) as mfpool,
            tc.tile_pool(name="rr", bufs=3) as rrpool,
            tc.tile_pool(name="ost", bufs=3) as ostpool,
            tc.tile_pool(name="p1", bufs=3, space="PSUM") as pp1,
            tc.tile_pool(name="p2", bufs=2, space="PSUM") as pp2,
        ):
            w1t = cpool.tile([128, 6, 2, 128], fp8)
            w2t = cpool.tile([128, 64], bf16)
            b1t = cpool.tile([128, 1], f32)
            nc.sync.dma_start(out=w1t[:], in_=w1_d[:])
            nc.sync.dma_start(out=w2t[:], in_=w2t_d[:])
            nc.sync.dma_start(out=b1t[:], in_=b1_d[:])

            def preload(b, s, first=False):
                """Issue all input loads + prefilter casts for strip (b, s).

                Everything serial here lives on Pool (SWDGE descgen, wrap
                columns, fp8 casts) or SP (mask load + broadcast), so it
                pipelines under the previous strip's PE/ACT/DVE compute.
                """
                r0 = s * STRIP
                xb = xbpool.tile([128, RB + 2, W + 2], bf16)
                # core rows 16*blk .. +16 -> rows 1..17 of tile
                # (split per channel: DMA AP balancer caps at 3 dims;
                #  SWDGE path casts fp32 -> bf16 in flight)
                for c in range(C):
                    nc.gpsimd.dma_start(
                        out=xb[c : 128 : C, 1 : RB + 1, 1 : W + 1],
                        in_=x_d[b, c, r0 : r0 + STRIP, :].rearrange(
                            "(blk r) w -> blk r w", blk=NBLK
                        ),
                    )
                # halo-top rows: r0 + 16*blk - 1 (wrap at image top)
                if r0 == 0:
                    nc.gpsimd.dma_start(
                        out=xb[8:128, 0, 1 : W + 1],
                        in_=x_d[b, :, RB - 1 : STRIP - RB : RB, :].rearrange(
                            "c k w -> k c w"
                        ),
                    )
                    nc.gpsimd.dma_start(
                        out=xb[0:8, 0, 1 : W + 1], in_=x_d[b, :, H - 1, :]
                    )
                else:
                    nc.gpsimd.dma_start(
                        out=xb[:, 0, 1 : W + 1],
                        in_=x_d[b, :, r0 - 1 : r0 + STRIP - RB : RB, :].rearrange(
                            "c k w -> k c w"
                        ),
                    )
                # halo-bottom rows: r0 + 16*blk + 16 (wrap at image bottom)
                if r0 + STRIP == H:
                    nc.gpsimd.dma_start(
                        out=xb[0:120, RB + 1, 1 : W + 1],
                        in_=x_d[b, :, r0 + RB : H - RB + 1 : RB, :].rearrange(
                            "c k w -> k c w"
                        ),
                    )
                    nc.gpsimd.dma_start(
                        out=xb[120:128, RB + 1, 1 : W + 1], in_=x_d[b, :, 0, :]
                    )
                else:
                    nc.gpsimd.dma_start(
                        out=xb[:, RB + 1, 1 : W + 1],
                        in_=x_d[b, :, r0 + RB : r0 + STRIP + 1 : RB, :].rearrange(
                            "c k w -> k c w"
                        ),
                    )
                # wrap columns + fp8 cast on Pool so they run right after
                # the loads instead of queueing behind DVE/ACT work (tile
                # emits EventSemaphore waits covering every SWDGE queue
                # before them, so the sync is complete)
                nc.gpsimd.tensor_copy(xb[:, :, 0], xb[:, :, W])
                nc.gpsimd.tensor_copy(xb[:, :, W + 1], xb[:, :, 1])
                xf8 = x8pool.tile([128, RB + 3, W + 2], fp8)
                nc.gpsimd.memset(xf8[:, RB + 2, :], 0.0)
                if first:
                    nc.scalar.copy(xf8[:, 0:5, :], xb[:, 0:5, :])
                    nc.vector.tensor_copy(xf8[:, 5:9, :], xb[:, 5:9, :])
                    nc.gpsimd.tensor_copy(xf8[:, 9:13, :], xb[:, 9:13, :])
                    nc.scalar.copy(xf8[:, 13 : RB + 2, :], xb[:, 13 : RB + 2, :])
                else:
                    nc.gpsimd.tensor_copy(xf8[:, 0 : RB + 2, :], xb[:])

                # mask: SWDGE cast-load i32 -> fp8 {0,1} grouped as
                # [16 part = blk, 16 rows x 512], then broadcast across the 8
                # channel partitions with partition-aligned SBUF->SBUF copies
                m8 = mkpool.tile([16, RB * W], fp8, tag="m8")
                nc.gpsimd.dma_start(
                    out=m8[:],
                    in_=m_d[b, 0, r0 : r0 + STRIP, :].rearrange(
                        "(p r) w -> p (r w)", p=16
                    ),
                )
                mfb = mfpool.tile([128, RB, W], fp8)
                mfv = mfb[:].rearrange("p r w -> p (r w)")
                for c in range(C):
                    nc.sync.dma_start(out=mfv[c : 128 : C], in_=m8[:])
                return xb, xf8, mfb

            strips = [(b, s) for b in range(BPC) for s in range(NSTRIP)]
            pre = preload(*strips[0], first=True)
            for si, (b, s) in enumerate(strips):
                xb, xf8, mfb = pre
                r0 = s * STRIP
                ov = o_d[b, :, r0 : r0 + STRIP, :].rearrange(
                    "c (blk half g) w -> half blk c (g w)", blk=NBLK, half=2, g=8
                )
                if si + 1 < len(strips):
                    pre = preload(*strips[si + 1])
                for half in range(2):
                        ost = ostpool.tile([128, 8, W], bf16)
                        for gg in range(8):
                            g = half * 8 + gg
                            p1 = pp1.tile([128, 2, W], f32)
                            for hf in range(2):
                                hp = slice(hf * 64, hf * 64 + 64)
                                for t, (ro, co, _) in enumerate(_tap_pairs()):
                                    if ro == 0:
                                        rhs = xf8[hp, g : g + 2, co : co + W]
                                    else:
                                        rhs = xf8[hp, g + 2 : g + 4, co : co + W]
                                    nc.tensor.matmul(
                                        p1[:, hf, :], w1t[hp, t, :, :], rhs,
                                        start=(t == 0), stop=(t == 5), perf_mode=DR,
                                    )
                            rr = rrpool.tile([128, 2, W], bf16)
                            nc.scalar.activation(rr[:], p1[:], Relu, bias=b1t[:, 0:1])
                            p2 = pp2.tile([128, W], f32)
                            nc.tensor.matmul(
                                p2[0:64, :], w2t[:], rr[:, 0, :], start=True, stop=True
                            )
                            nc.tensor.matmul(
                                p2[64:128, :], w2t[:], rr[:, 1, :],
                                start=True, stop=True, tile_position=(0, 64),
                            )
                            # out = delta*mask + x   (mask bcast is fp8 {0,1})
                            nc.vector.tensor_mul(
                                ost[:, gg, :], p2[:], mfb[:, g, :]
                            )
                            nc.vector.tensor_add(
                                ost[:, gg, :], ost[:, gg, :], xb[:, g + 1, 1 : W + 1]
                            )
                            if gg == 3:
                                nc.sync.dma_start(
                                    out=ov[half][:, :, 0 : 4 * W], in_=ost[:, 0:4, :]
                                )
                        nc.sync.dma_start(
                            out=ov[half][:, :, 4 * W : 8 * W], in_=ost[:, 4:8, :]
                        )
    nc.compile()
    return nc


def _get_nc():
    if "nc" not in _CACHE:
        _CACHE["nc"] = _build_bass()
    return _CACHE["nc"]


def _fold_weights(w1_w, w1_b, w2_w):
    bf = ml_dtypes.bfloat16
    e4 = ml_dtypes.float8_e4m3
    filt = _fixed_filters()  # [4,3,3] float64
    w1r = w1_w.astype(np.float64).reshape(HID, C, 4)
    weff = np.einsum("ocf,fij->ocij", w1r, filt)  # [16,8,3,3]

    # per-hid-channel fp8 scale, folded into w2 and the relu bias
    s_o = np.abs(weff).max(axis=(1, 2, 3)) / 240.0
    weff_s = weff / s_o[:, None, None, None]

    w1dr = np.zeros((128, 6, 2, 128), np.float64)
    for t, (ro, co, taps) in enumerate(_tap_pairs()):
        for i, tap in enumerate(taps):
            if tap is None:
                continue
            dy, dx, frac = tap
            for blk in range(8):
                for c in range(C):
                    w1dr[blk * 8 + c, t, i, blk * 16 : blk * 16 + 16] = (
                        weff_s[:, c, dy, dx] * frac
                    )
    w1dr[64:128] = w1dr[0:64]
    w1dr8 = w1dr.astype(e4)

    w2t = np.zeros((128, 64), np.float64)
    for blk in range(8):
        for hid in range(HID):
            for co in range(C):
                w2t[blk * 16 + hid, blk * 8 + co] = w2_w[co, hid] * s_o[hid]

    b1 = np.zeros((128, 1), np.float32)
    for blk in range(8):
        b1[blk * 16 : blk * 16 + 16, 0] = (w1_b / s_o).astype(np.float32)

    return (
        np.ascontiguousarray(w1dr8),
        np.ascontiguousarray(w2t.astype(bf)),
        b1,
    )


def kernel(x, w1_w, w1_b, w2_w, update_mask):
    from concourse.bass_utils import run_bass_kernel_spmd

    x = np.ascontiguousarray(np.asarray(x), dtype=np.float32)
    update_mask = np.ascontiguousarray(np.asarray(update_mask), dtype=np.int32)
    w1dr, w2t, b1 = _fold_weights(
        np.asarray(w1_w, np.float64), np.asarray(w1_b, np.float64),
        np.asarray(w2_w, np.float64),
    )

    nc = _get_nc()
    in_maps = []
    for i in range(NCORES):
        in_maps.append(
            {
                "x": np.ascontiguousarray(x[i * BPC : (i + 1) * BPC]),
                "update_mask": np.ascontiguousarray(
                    update_mask[i * BPC : (i + 1) * BPC]
                ),
                "w1dr": w1dr,
                "w2t": w2t,
                "bias1": b1,
            }
        )
    res = run_bass_kernel_spmd(nc, in_maps, core_ids=list(range(NCORES)))
    out = np.concatenate(
        [np.asarray(r["out"], dtype=np.float32) for r in res.results], axis=0
    )
    return out
